# revision 29
# baseline (speedup 1.0000x reference)
"""Trainium2 Bass kernel for CVectorQuantiser (VQ codebook lookup).

Problem: z [16,256,32,32] f32, weight [8192,256] f32 (l2-normalized rows).
  zc  = l2norm(z tokens)              # 16384 tokens of dim 256
  d   = 2*zc@w.T - |w_k|^2 (+const)   # [16384, 8192]
  idx = argmax_k d                    # first-max, like jnp.argmax
  z_q = w[idx] (straight-through: zc + (w[idx]-zc)), loss = beta*mean((w[idx]-zc)^2)

Sharding: data-parallel over tokens. 8 cores x 2048 tokens (2 'b' images each),
codebook replicated. Host only concatenates shards and sums 8 loss partials.

Per-core dataflow (all fp32 unless noted):
  - z load + normalize: sumsq via PE ones-reduce, ACT sqrt(0.25*ss), DVE
    reciprocal -> inv2 = 2/norm; PE-broadcast row; zc2 = z*inv2 (= 2*zc).
  - one-time: PE-transpose weight -> wT [256d x 8192k] in SBUF; wsq row
    [1,8192]; negdelta = (1 - wsq) as bf16 row (|delta|<2e-5 so bf16
    quantization error ~6e-8, below fp32 noise; the -1 constant is
    per-token-irrelevant for argmax).
  - scores: 16 token-tiles x 8 units of 1024 codes: PE matmul group per unit:
    bf16 K=1 matmul accumulates -delta, then 2 fp32 d-chunk matmuls
    (lhsT=zc2 tile [128d,128t], rhs=wT [128d,512k]) -> PSUM [128,1024]
    already wsq-corrected. DVE 3D-reduce PSUM -> per-512-chunk maxes;
    ACT copies PSUM -> scores row [128, 8192] in SBUF.
  - argmax: per tile max8(chunk maxes [128,16]) -> max_index over scores row.
  - gather: idx -> int16 wrapped layout (DRAM bounce), gpsimd ap_gather from wT.
  - outputs: diff = wq - zc; loss partial = ACT Square+accum; z_q = zc + diff.
"""

import sys
import numpy as np

sys.path.insert(0, "/opt/trn_rl_repo")

B_FULL, C, H, W = 16, 256, 32, 32
HW = H * W                      # 1024
N_CORES = 8
B_LOC = B_FULL // N_CORES       # 2 images per core
T = B_LOC * HW                  # 2048 tokens per core
K = 8192                        # codebook size
D = 256                         # code dim
N_TILES = T // 128              # 16 token tiles
N_UNITS = 8                     # units of K per tile (1024 codes each)
UK = K // N_UNITS               # 1024
BETA = 0.25

_CACHE = {}


class _PhaseDone(Exception):
    pass


import contextlib as _ctxlib


@_ctxlib.contextmanager
def _phase_guard():
    try:
        yield
    except _PhaseDone:
        pass


def _build_program():
    import os
    PHASE = int(os.environ.get("KPHASE", "99"))
    import concourse.bacc as bacc
    import concourse.tile as tile
    import concourse.mybir as mybir

    f32 = mybir.dt.float32
    bf16 = mybir.dt.bfloat16
    i32 = mybir.dt.int32
    u32 = mybir.dt.uint32
    i16 = mybir.dt.int16
    X = mybir.AxisListType.X
    Alu = mybir.AluOpType
    Act = mybir.ActivationFunctionType

    nc = bacc.Bacc("TRN2", target_bir_lowering=False, debug=False)

    z_in = nc.dram_tensor("z", [B_LOC, C, HW], f32, kind="ExternalInput").ap()
    w_in = nc.dram_tensor("weight", [K, D], f32, kind="ExternalInput").ap()
    id_in = nc.dram_tensor("id128", [128, 128], f32, kind="ExternalInput").ap()
    zq_out = nc.dram_tensor("z_q", [B_LOC, C, HW], f32, kind="ExternalOutput").ap()
    idx_out = nc.dram_tensor("idx", [T], i32, kind="ExternalOutput").ap()
    loss_out = nc.dram_tensor("loss_sum", [1, 1], f32, kind="ExternalOutput").ap()
    idx_bounce = nc.dram_tensor("idx_bounce", [T], i16).ap()
    nd_bounce = nc.dram_tensor("nd_bounce", [K], bf16).ap()

    import contextlib
    with tile.TileContext(nc) as tc:
        with contextlib.ExitStack() as ctx ,_phase_guard():
            # ---------------- pools ----------------
            big = ctx.enter_context(tc.tile_pool(name="big", bufs=1))
            sc_pool = ctx.enter_context(tc.tile_pool(name="scores", bufs=1))
            ztmp_pool = ctx.enter_context(tc.tile_pool(name="ztmp", bufs=1))
            wtmp_pool = ctx.enter_context(tc.tile_pool(name="wtmp", bufs=3))
            row_pool = ctx.enter_context(tc.tile_pool(name="rows", bufs=2))
            small = ctx.enter_context(tc.tile_pool(name="small", bufs=1))
            tiny = ctx.enter_context(tc.tile_pool(name="tiny", bufs=4))
            psum = ctx.enter_context(tc.tile_pool(name="ps", bufs=4, space="PSUM"))

            # ---------------- fixed tiles ----------------
            wT = big.tile([128, 2 * K], f32)          # [d%128, dc*K + k]
            zc2 = big.tile([128, 2 * T], f32)         # 2*zc   [dc*T + t]
            wq = big.tile([128, 2 * T], f32)          # gathered codes (then diff)
            zraw = big.tile([128, 2 * T], f32)        # raw z (later z_q)
            negdelta = big.tile([1, K], bf16)         # 1 - |w_k|^2, bf16
            idx_all = big.tile([128, N_TILES], u32)
            idx_all16 = big.tile([128, N_TILES], i16)
            idxs_w = big.tile([128, 128], i16)        # wrapped idx for gathers
            id128 = big.tile([128, 128], f32)
            ones_col = big.tile([128, 1], f32)
            ones_row = big.tile([1, 128], f32)
            ones_row_bf = big.tile([1, 128], bf16)
            lpart_all = big.tile([128, 4], f32)
            loss_sb = big.tile([1, 1], f32)

            scores = sc_pool.tile([128, K], f32)      # one token-tile's scores

            nc.vector.memset(ones_col[:], 1.0)
            nc.vector.memset(ones_row[:], 1.0)
            nc.vector.memset(ones_row_bf[:], 1.0)
            nc.sync.dma_start(out=id128[:], in_=id_in[:])

            # ---------------- load z (vector queue) + weight (sync queue) ---
            for b in range(B_LOC):
                for dc in range(2):
                    nc.gpsimd.dma_start(
                        out=zraw[:, dc * T + b * HW: dc * T + (b + 1) * HW],
                        in_=z_in[b, dc * 128:(dc + 1) * 128, :],
                    )

            # ---------------- normalize: zc2 = 2*z/||z|| ----------------
            for b in range(B_LOC):
                zsq = ztmp_pool.tile([128, HW], f32, tag="zsq")
                ps_ss = psum.tile([128, UK], f32, tag="ps")
                for dc in range(2):
                    nc.vector.tensor_tensor(
                        zsq[:],
                        zraw[:, dc * T + b * HW: dc * T + (b + 1) * HW],
                        zraw[:, dc * T + b * HW: dc * T + (b + 1) * HW],
                        Alu.mult,
                    )
                    for nb in range(2):
                        nc.tensor.matmul(
                            ps_ss[0:1, nb * 512:(nb + 1) * 512],
                            ones_col[:],
                            zsq[:, nb * 512:(nb + 1) * 512],
                            start=(dc == 0), stop=(dc == 1),
                        )
                snorm_row = small.tile([1, HW], f32, tag="snorm")
                inv2_row = small.tile([1, HW], f32, tag="inv2")
                nc.scalar.activation(
                    snorm_row[:], ps_ss[0:1, 0:HW], Act.Sqrt, scale=0.25,
                )
                nc.vector.reciprocal(inv2_row[:], snorm_row[:])
                ps_bc2 = psum.tile([128, UK], f32, tag="ps")
                for nb in range(2):
                    nc.tensor.matmul(
                        ps_bc2[:, nb * 512:(nb + 1) * 512],
                        ones_row[:],
                        inv2_row[:, nb * 512:(nb + 1) * 512],
                        start=True, stop=True,
                    )
                for dc in range(2):
                    nc.vector.tensor_tensor(
                        zc2[:, dc * T + b * HW: dc * T + (b + 1) * HW],
                        zraw[:, dc * T + b * HW: dc * T + (b + 1) * HW],
                        ps_bc2[:, 0:HW],
                        Alu.mult,
                    )

            if PHASE < 1:
                raise _PhaseDone()
            # ------- weight transpose: wT[d, k]; wsq via ACT accum -------
            # load in bands of 4 kb-blocks (512 codebook rows) per DMA:
            # wtmp[p, g*D + d] = weight[band*512 + g*128 + p, d]
            wsqcol = row_pool.tile([128, K // 128], f32, tag="wsqcol")
            w_band = w_in.rearrange("(a g p) d -> a p g d", g=4, p=128)
            for band in range(K // 512):
                wtmp = wtmp_pool.tile([128, 4 * D], f32, tag="wtmp")
                dma_eng = nc.sync if band % 2 == 0 else nc.scalar
                dma_eng.dma_start(
                    out=wtmp.rearrange("p (g d) -> p g d", g=4),
                    in_=w_band[band],
                )
                for g in range(4):
                    kb = band * 4 + g
                    for dc in range(2):
                        pt = psum.tile([128, UK], f32, tag="ps")
                        nc.tensor.transpose(
                            pt[:, 0:128],
                            wtmp[:, g * D + dc * 128: g * D + (dc + 1) * 128],
                            id128[:],
                        )
                        nc.vector.tensor_copy(
                            wT[:, dc * K + kb * 128: dc * K + (kb + 1) * 128],
                            pt[:, 0:128],
                        )
                    sq_scr = ztmp_pool.tile([128, D], f32, tag="sqscr")
                    nc.scalar.activation(
                        sq_scr[:], wtmp[:, g * D:(g + 1) * D], Act.Square,
                        accum_out=wsqcol[:, kb:kb + 1],
                    )

            # negdelta = (1 - wsq) bf16, via column layout + DRAM bounce.
            # Split in halves so score tiles can start before full setup.
            ndcol = row_pool.tile([128, K // 128], bf16, tag="ndcol")
            ndb = nd_bounce.rearrange("(h j p) -> h p j", h=2, p=128)
            ndr = nd_bounce.rearrange("(h o k) -> h o k", h=2, o=1)
            for h in range(2):
                cols = slice(h * (K // 256), (h + 1) * (K // 256))
                nc.vector.tensor_scalar(
                    ndcol[:, cols], wsqcol[:, cols], -1.0, 1.0,
                    op0=Alu.mult, op1=Alu.add,
                )
                nc.gpsimd.dma_start(out=ndb[h], in_=ndcol[:, cols])
                nc.gpsimd.dma_start(
                    out=negdelta[:, h * (K // 2):(h + 1) * (K // 2)], in_=ndr[h]
                )

            if PHASE < 2:
                raise _PhaseDone()
            # ---------------- main: scores + argmax ----------------
            TH = HW  # tokens per half(-image)
            ib = idx_bounce.rearrange("(b j p) -> b p j", b=2, p=128)
            iw = idx_bounce.rearrange("(b s q) -> b q s", b=2, q=16)
            io = idx_out.rearrange("(b j p) -> b p j", b=2, p=128)
            zc = zc2   # halved in place per half once its matmuls are done
            diff = wq
            zq_sb = zraw

            def epilogue_half(b):
                cols = slice(8 * b, 8 * b + 8)
                nc.vector.tensor_copy(idx_all16[:, cols], idx_all[:, cols])
                nc.gpsimd.dma_start(out=ib[b], in_=idx_all16[:, cols])
                idxs_w_b = idxs_w[:, 64 * b:64 * (b + 1)]
                for g in range(8):
                    nc.gpsimd.dma_start(
                        out=idxs_w_b[16 * g:16 * (g + 1), :], in_=iw[b]
                    )
                nc.gpsimd.dma_start(out=io[b], in_=idx_all[:, cols].bitcast(i32))
                for dc in range(2):
                    sl = slice(dc * T + b * HW, dc * T + (b + 1) * HW)
                    nc.gpsimd.ap_gather(
                        wq[:, sl],
                        wT[:, dc * K:(dc + 1) * K],
                        idxs_w_b[:],
                        channels=128, num_elems=K, d=1, num_idxs=TH,
                    )
                    nc.vector.tensor_scalar_mul(zc[:, sl], zc2[:, sl], 0.5)
                    nc.vector.tensor_tensor(diff[:, sl], wq[:, sl], zc[:, sl],
                                            Alu.subtract)
                    nc.vector.tensor_tensor(zq_sb[:, sl], zc[:, sl],
                                            diff[:, sl], Alu.add)
                    nc.sync.dma_start(
                        out=zq_out[b, dc * 128:(dc + 1) * 128, :],
                        in_=zq_sb[:, sl],
                    )
                    # loss partial via ACT Square + accum (scratch: ztmp pool)
                    lsc = ztmp_pool.tile([128, HW], f32, tag="zsq")
                    nc.scalar.activation(
                        lsc[:], diff[:, sl], Act.Square,
                        accum_out=lpart_all[:, 2 * b + dc: 2 * b + dc + 1],
                    )

            for t in range(N_TILES):
                cmax = tiny.tile([128, 16], f32, tag="cmax")
                for u in range(N_UNITS):
                    ps_u = psum.tile([128, UK], f32, tag="ps")
                    # -delta via bf16 K=1 matmul (starts the accumulation)
                    for nb in range(2):
                        nc.tensor.matmul(
                            ps_u[:, nb * 512:(nb + 1) * 512],
                            ones_row_bf[:],
                            negdelta[:, u * UK + nb * 512: u * UK + (nb + 1) * 512],
                            start=True, stop=False, skip_group_check=True,
                        )
                    for dc in range(2):
                        lhsT = zc2[:, dc * T + t * 128: dc * T + (t + 1) * 128]
                        for nb in range(2):
                            nc.tensor.matmul(
                                ps_u[:, nb * 512:(nb + 1) * 512],
                                lhsT,
                                wT[:, dc * K + u * UK + nb * 512:
                                   dc * K + u * UK + (nb + 1) * 512],
                                start=False, stop=(dc == 1),
                                skip_group_check=True,
                            )
                    # per-512-chunk maxes straight from PSUM
                    nc.vector.tensor_reduce(
                        cmax[:, 2 * u:2 * u + 2],
                        ps_u.rearrange("p (c g) -> p c g", c=2),
                        axis=X, op=Alu.max,
                    )
                    # corrected scores PSUM -> SBUF row (ACT)
                    nc.scalar.activation(
                        scores[:, u * UK:(u + 1) * UK], ps_u[:], Act.Copy,
                    )
                m8 = tiny.tile([128, 8], f32, tag="m8")
                i8 = tiny.tile([128, 8], u32, tag="i8")
                nc.vector.max(m8[:], cmax[:])
                nc.vector.max_index(i8[:], m8[:], scores[:])
                nc.vector.tensor_copy(idx_all[:, t:t + 1], i8[:, 0:1])
                if t == N_TILES // 2 - 1:
                    if PHASE < 3:
                        raise _PhaseDone()
                    epilogue_half(0)
                elif t == N_TILES - 1:
                    epilogue_half(1)

            lsum = big.tile([128, 1], f32)
            nc.vector.tensor_reduce(lsum[:], lpart_all[:], axis=X, op=Alu.add)
            ps_l = psum.tile([128, UK], f32, tag="ps")
            nc.tensor.matmul(ps_l[0:1, 0:1], lsum[:], ones_col[:],
                             start=True, stop=True)
            nc.scalar.activation(loss_sb[:], ps_l[0:1, 0:1], Act.Copy)
            nc.sync.dma_start(out=loss_out[:], in_=loss_sb[:])

    nc.compile()
    return nc


def _get_program():
    if "nc" not in _CACHE:
        _CACHE["nc"] = _build_program()
    return _CACHE["nc"]


def kernel(z: np.ndarray, weight: np.ndarray):
    from concourse.bass_utils import run_bass_kernel_spmd

    nc = _get_program()
    z = np.ascontiguousarray(z, dtype=np.float32).reshape(B_FULL, C, HW)
    weight = np.ascontiguousarray(weight, dtype=np.float32)
    id128 = np.eye(128, dtype=np.float32)

    in_maps = []
    for c in range(N_CORES):
        in_maps.append({
            "z": z[c * B_LOC:(c + 1) * B_LOC],
            "weight": weight,
            "id128": id128,
        })
    res = run_bass_kernel_spmd(nc, in_maps, list(range(N_CORES)))

    zq = np.concatenate(
        [res.results[c]["z_q"].reshape(B_LOC, C, H, W) for c in range(N_CORES)],
        axis=0,
    )
    idx = np.concatenate([res.results[c]["idx"] for c in range(N_CORES)])
    total = np.sum([np.float64(res.results[c]["loss_sum"][0, 0])
                    for c in range(N_CORES)])
    loss = np.float32(BETA * total / (B_FULL * HW * C))
    return zq, loss, idx.astype(np.int32)


# revision 35
# speedup vs baseline: 4267.5356x; 4267.5356x over previous
"""Trainium2 Bass kernel for CVectorQuantiser (VQ codebook lookup).

Problem: z [16,256,32,32] f32, weight [8192,256] f32 (l2-normalized rows).
  zc  = l2norm(z tokens)              # 16384 tokens of dim 256
  d   = 2*zc@w.T - |w_k|^2 (+const)   # [16384, 8192]
  idx = argmax_k d                    # first-max, like jnp.argmax
  z_q = w[idx] (straight-through: zc + (w[idx]-zc)), loss = beta*mean((w[idx]-zc)^2)

Sharding: data-parallel over tokens. 8 cores x 2048 tokens (2 'b' images each),
codebook replicated. Host only concatenates shards and sums 8 loss partials.

Per-core dataflow (all fp32 unless noted):
  - z load + normalize: sumsq via PE ones-reduce, ACT sqrt(0.25*ss), DVE
    reciprocal -> inv2 = 2/norm; PE-broadcast row; zc2 = z*inv2 (= 2*zc).
  - one-time: PE-transpose weight -> wT [256d x 8192k] in SBUF; wsq row
    [1,8192]; negdelta = (1 - wsq) as bf16 row (|delta|<2e-5 so bf16
    quantization error ~6e-8, below fp32 noise; the -1 constant is
    per-token-irrelevant for argmax).
  - scores: 16 token-tiles x 8 units of 1024 codes: PE matmul group per unit:
    bf16 K=1 matmul accumulates -delta, then 2 fp32 d-chunk matmuls
    (lhsT=zc2 tile [128d,128t], rhs=wT [128d,512k]) -> PSUM [128,1024]
    already wsq-corrected. DVE 3D-reduce PSUM -> per-512-chunk maxes;
    ACT copies PSUM -> scores row [128, 8192] in SBUF.
  - argmax: per tile max8(chunk maxes [128,16]) -> max_index over scores row.
  - gather: idx -> int16 wrapped layout (DRAM bounce), gpsimd ap_gather from wT.
  - outputs: diff = wq - zc; loss partial = ACT Square+accum; z_q = zc + diff.
"""

import sys
import numpy as np

sys.path.insert(0, "/opt/trn_rl_repo")

B_FULL, C, H, W = 16, 256, 32, 32
HW = H * W                      # 1024
N_CORES = 8
B_LOC = B_FULL // N_CORES       # 2 images per core
T = B_LOC * HW                  # 2048 tokens per core
K = 8192                        # codebook size
D = 256                         # code dim
N_TILES = T // 128              # 16 token tiles
N_UNITS = 8                     # units of K per tile (1024 codes each)
UK = K // N_UNITS               # 1024
BETA = 0.25

_CACHE = {}


class _PhaseDone(Exception):
    pass


import contextlib as _ctxlib


@_ctxlib.contextmanager
def _phase_guard():
    try:
        yield
    except _PhaseDone:
        pass


def _build_program():
    import os
    PHASE = int(os.environ.get("KPHASE", "99"))
    import concourse.bacc as bacc
    import concourse.tile as tile
    import concourse.mybir as mybir

    f32 = mybir.dt.float32
    bf16 = mybir.dt.bfloat16
    i32 = mybir.dt.int32
    u32 = mybir.dt.uint32
    i16 = mybir.dt.int16
    X = mybir.AxisListType.X
    Alu = mybir.AluOpType
    Act = mybir.ActivationFunctionType

    nc = bacc.Bacc("TRN2", target_bir_lowering=False, debug=False)

    z_in = nc.dram_tensor("z", [B_LOC, C, HW], f32, kind="ExternalInput").ap()
    w_in = nc.dram_tensor("weight", [K, D], f32, kind="ExternalInput").ap()
    id_in = nc.dram_tensor("id128", [128, 128], f32, kind="ExternalInput").ap()
    zq_out = nc.dram_tensor("z_q", [B_LOC, C, HW], f32, kind="ExternalOutput").ap()
    idx_out = nc.dram_tensor("idx", [T], i32, kind="ExternalOutput").ap()
    loss_out = nc.dram_tensor("loss_sum", [1, 1], f32, kind="ExternalOutput").ap()
    idx_bounce = nc.dram_tensor("idx_bounce", [T], i16).ap()
    nd_bounce = nc.dram_tensor("nd_bounce", [K], bf16).ap()

    import contextlib
    with tile.TileContext(nc) as tc:
        with contextlib.ExitStack() as ctx ,_phase_guard():
            # ---------------- pools ----------------
            big = ctx.enter_context(tc.tile_pool(name="big", bufs=1))
            sc_pool = ctx.enter_context(tc.tile_pool(name="scores", bufs=1))
            ztmp_pool = ctx.enter_context(tc.tile_pool(name="ztmp", bufs=1))
            wtmp_pool = ctx.enter_context(tc.tile_pool(name="wtmp", bufs=5))
            row_pool = ctx.enter_context(tc.tile_pool(name="rows", bufs=2))
            small = ctx.enter_context(tc.tile_pool(name="small", bufs=1))
            tiny = ctx.enter_context(tc.tile_pool(name="tiny", bufs=6))
            psum = ctx.enter_context(tc.tile_pool(name="ps", bufs=4, space="PSUM"))

            # ---------------- fixed tiles ----------------
            wT = big.tile([128, 2 * K], f32)          # [d%128, dc*K + k]
            zc2 = big.tile([128, 2 * T], f32)         # 2*zc   [dc*T + t]
            wq = big.tile([128, 2 * T], f32)          # gathered codes (then diff)
            zraw = big.tile([128, 2 * T], f32)        # raw z (later z_q)
            negdelta = big.tile([1, K], bf16)         # 1 - |w_k|^2, bf16
            idx_all = big.tile([128, N_TILES], u32)
            idx_all16 = big.tile([128, N_TILES], i16)
            idxs_w = big.tile([128, 128], i16)        # wrapped idx for gathers
            id128 = big.tile([128, 128], f32)
            ones_col = big.tile([128, 1], f32)
            ones_row = big.tile([1, 128], f32)
            ones_row_bf = big.tile([1, 128], bf16)
            lpart_all = big.tile([128, 4], f32)
            loss_sb = big.tile([1, 1], f32)

            scores = sc_pool.tile([128, K], f32)      # one token-tile's scores

            nc.vector.memset(ones_col[:], 1.0)
            nc.vector.memset(ones_row[:], 1.0)
            nc.vector.memset(ones_row_bf[:], 1.0)
            nc.sync.dma_start(out=id128[:], in_=id_in[:])

            # ---------------- load z (vector queue) + weight (sync queue) ---
            for b in range(B_LOC):
                for dc in range(2):
                    nc.gpsimd.dma_start(
                        out=zraw[:, dc * T + b * HW: dc * T + (b + 1) * HW],
                        in_=z_in[b, dc * 128:(dc + 1) * 128, :],
                    )

            # ---------------- normalize: zc2 = 2*z/||z|| ----------------
            for b in range(B_LOC):
                zsq = ztmp_pool.tile([128, HW], f32, tag="zsq")
                ps_ss = psum.tile([128, UK], f32, tag="ps")
                for dc in range(2):
                    nc.vector.tensor_tensor(
                        zsq[:],
                        zraw[:, dc * T + b * HW: dc * T + (b + 1) * HW],
                        zraw[:, dc * T + b * HW: dc * T + (b + 1) * HW],
                        Alu.mult,
                    )
                    for nb in range(2):
                        nc.tensor.matmul(
                            ps_ss[0:1, nb * 512:(nb + 1) * 512],
                            ones_col[:],
                            zsq[:, nb * 512:(nb + 1) * 512],
                            start=(dc == 0), stop=(dc == 1),
                        )
                snorm_row = small.tile([1, HW], f32, tag="snorm")
                inv2_row = small.tile([1, HW], f32, tag="inv2")
                nc.scalar.activation(
                    snorm_row[:], ps_ss[0:1, 0:HW], Act.Sqrt, scale=0.25,
                )
                nc.vector.reciprocal(inv2_row[:], snorm_row[:])
                ps_bc2 = psum.tile([128, UK], f32, tag="ps")
                for nb in range(2):
                    nc.tensor.matmul(
                        ps_bc2[:, nb * 512:(nb + 1) * 512],
                        ones_row[:],
                        inv2_row[:, nb * 512:(nb + 1) * 512],
                        start=True, stop=True,
                    )
                for dc in range(2):
                    nc.vector.tensor_tensor(
                        zc2[:, dc * T + b * HW: dc * T + (b + 1) * HW],
                        zraw[:, dc * T + b * HW: dc * T + (b + 1) * HW],
                        ps_bc2[:, 0:HW],
                        Alu.mult,
                    )

            if PHASE < 1:
                raise _PhaseDone()
            # ------- weight transpose: wT[d, k]; wsq via ACT accum -------
            # load in bands of 4 kb-blocks (512 codebook rows) per DMA:
            # wtmp[p, g*D + d] = weight[band*512 + g*128 + p, d]
            wsqcol = row_pool.tile([128, K // 128], f32, tag="wsqcol")
            w_band = w_in.rearrange("(a g p) d -> a p g d", g=4, p=128)
            for band in range(K // 512):
                wtmp = wtmp_pool.tile([128, 4 * D], f32, tag="wtmp")
                dma_eng = nc.sync if band % 2 == 0 else nc.scalar
                dma_eng.dma_start(
                    out=wtmp.rearrange("p (g d) -> p g d", g=4),
                    in_=w_band[band],
                )
                for g in range(4):
                    kb = band * 4 + g
                    for dc in range(2):
                        pt = psum.tile([128, UK], f32, tag="ps")
                        nc.tensor.transpose(
                            pt[:, 0:128],
                            wtmp[:, g * D + dc * 128: g * D + (dc + 1) * 128],
                            id128[:],
                        )
                        nc.vector.tensor_copy(
                            wT[:, dc * K + kb * 128: dc * K + (kb + 1) * 128],
                            pt[:, 0:128],
                        )
                    sq_scr = ztmp_pool.tile([128, D], f32, tag="sqscr")
                    nc.scalar.activation(
                        sq_scr[:], wtmp[:, g * D:(g + 1) * D], Act.Square,
                        accum_out=wsqcol[:, kb:kb + 1],
                    )

            # negdelta = (1 - wsq) bf16, via column layout + DRAM bounce.
            # Split in halves so score tiles can start before full setup.
            ndcol = row_pool.tile([128, K // 128], bf16, tag="ndcol")
            ndb = nd_bounce.rearrange("(h j p) -> h p j", h=8, p=128)
            ndr = nd_bounce.rearrange("(h o k) -> h o k", h=8, o=1)
            for h in range(8):
                cols = slice(h * (K // 1024), (h + 1) * (K // 1024))
                nc.vector.tensor_scalar(
                    ndcol[:, cols], wsqcol[:, cols], -1.0, 1.0,
                    op0=Alu.mult, op1=Alu.add,
                )
                nc.gpsimd.dma_start(out=ndb[h], in_=ndcol[:, cols])
                nc.gpsimd.dma_start(
                    out=negdelta[:, h * (K // 8):(h + 1) * (K // 8)], in_=ndr[h]
                )

            if PHASE < 2:
                raise _PhaseDone()
            # ---------------- main: scores + argmax ----------------
            TH = HW  # tokens per half(-image)
            ib = idx_bounce.rearrange("(b j p) -> b p j", b=2, p=128)
            iw = idx_bounce.rearrange("(b s q) -> b q s", b=2, q=16)
            io = idx_out.rearrange("(b j p) -> b p j", b=2, p=128)
            zc = zc2   # halved in place per half once its matmuls are done
            diff = wq
            zq_sb = zraw

            def epilogue_half(b):
                cols = slice(8 * b, 8 * b + 8)
                nc.vector.tensor_copy(idx_all16[:, cols], idx_all[:, cols])
                nc.gpsimd.dma_start(out=ib[b], in_=idx_all16[:, cols])
                idxs_w_b = idxs_w[:, 64 * b:64 * (b + 1)]
                for g in range(8):
                    nc.gpsimd.dma_start(
                        out=idxs_w_b[16 * g:16 * (g + 1), :], in_=iw[b]
                    )
                nc.gpsimd.dma_start(out=io[b], in_=idx_all[:, cols].bitcast(i32))
                for dc in range(2):
                    sl = slice(dc * T + b * HW, dc * T + (b + 1) * HW)
                    nc.gpsimd.ap_gather(
                        wq[:, sl],
                        wT[:, dc * K:(dc + 1) * K],
                        idxs_w_b[:],
                        channels=128, num_elems=K, d=1, num_idxs=TH,
                    )
                    nc.vector.tensor_scalar_mul(zc[:, sl], zc2[:, sl], 0.5)
                    nc.vector.tensor_tensor(diff[:, sl], wq[:, sl], zc[:, sl],
                                            Alu.subtract)
                    nc.vector.tensor_tensor(zq_sb[:, sl], zc[:, sl],
                                            diff[:, sl], Alu.add)
                    nc.sync.dma_start(
                        out=zq_out[b, dc * 128:(dc + 1) * 128, :],
                        in_=zq_sb[:, sl],
                    )
                    # loss partial via ACT Square + accum (scratch: ztmp pool)
                    lsc = ztmp_pool.tile([128, HW], f32, tag="zsq")
                    nc.scalar.activation(
                        lsc[:], diff[:, sl], Act.Square,
                        accum_out=lpart_all[:, 2 * b + dc: 2 * b + dc + 1],
                    )

            for t in range(N_TILES):
                cmax = tiny.tile([128, 16], f32, tag="cmax")
                for u in range(N_UNITS):
                    ps_u = psum.tile([128, UK], f32, tag="ps")
                    # -delta via bf16 K=1 matmul (starts the accumulation)
                    for nb in range(2):
                        nc.tensor.matmul(
                            ps_u[:, nb * 512:(nb + 1) * 512],
                            ones_row_bf[:],
                            negdelta[:, u * UK + nb * 512: u * UK + (nb + 1) * 512],
                            start=True, stop=False, skip_group_check=True,
                        )
                    for dc in range(2):
                        lhsT = zc2[:, dc * T + t * 128: dc * T + (t + 1) * 128]
                        for nb in range(2):
                            nc.tensor.matmul(
                                ps_u[:, nb * 512:(nb + 1) * 512],
                                lhsT,
                                wT[:, dc * K + u * UK + nb * 512:
                                   dc * K + u * UK + (nb + 1) * 512],
                                start=False, stop=(dc == 1),
                                skip_group_check=True,
                            )
                    # per-512-chunk maxes straight from PSUM
                    nc.vector.tensor_reduce(
                        cmax[:, 2 * u:2 * u + 2],
                        ps_u.rearrange("p (c g) -> p c g", c=2),
                        axis=X, op=Alu.max,
                    )
                    # corrected scores PSUM -> SBUF row (ACT)
                    nc.scalar.activation(
                        scores[:, u * UK:(u + 1) * UK], ps_u[:], Act.Copy,
                    )
                m8 = tiny.tile([128, 8], f32, tag="m8")
                i8 = tiny.tile([128, 8], u32, tag="i8")
                nc.vector.max(m8[:], cmax[:])
                nc.vector.max_index(i8[:], m8[:], scores[:])
                nc.vector.tensor_copy(idx_all[:, t:t + 1], i8[:, 0:1])
                if t == N_TILES // 2 - 1:
                    if PHASE < 3:
                        raise _PhaseDone()
                    epilogue_half(0)
                elif t == N_TILES - 1:
                    epilogue_half(1)

            lsum = big.tile([128, 1], f32)
            nc.vector.tensor_reduce(lsum[:], lpart_all[:], axis=X, op=Alu.add)
            ps_l = psum.tile([128, UK], f32, tag="ps")
            nc.tensor.matmul(ps_l[0:1, 0:1], lsum[:], ones_col[:],
                             start=True, stop=True)
            nc.scalar.activation(loss_sb[:], ps_l[0:1, 0:1], Act.Copy)
            nc.sync.dma_start(out=loss_out[:], in_=loss_sb[:])

    nc.compile()
    return nc


def _get_program():
    if "nc" not in _CACHE:
        _CACHE["nc"] = _build_program()
    return _CACHE["nc"]


def kernel(z: np.ndarray, weight: np.ndarray):
    from concourse.bass_utils import run_bass_kernel_spmd

    nc = _get_program()
    z = np.ascontiguousarray(z, dtype=np.float32).reshape(B_FULL, C, HW)
    weight = np.ascontiguousarray(weight, dtype=np.float32)
    id128 = np.eye(128, dtype=np.float32)

    in_maps = []
    for c in range(N_CORES):
        in_maps.append({
            "z": z[c * B_LOC:(c + 1) * B_LOC],
            "weight": weight,
            "id128": id128,
        })
    res = run_bass_kernel_spmd(nc, in_maps, list(range(N_CORES)))

    zq = np.concatenate(
        [res.results[c]["z_q"].reshape(B_LOC, C, H, W) for c in range(N_CORES)],
        axis=0,
    )
    idx = np.concatenate([res.results[c]["idx"] for c in range(N_CORES)])
    total = np.sum([np.float64(res.results[c]["loss_sum"][0, 0])
                    for c in range(N_CORES)])
    loss = np.float32(BETA * total / (B_FULL * HW * C))
    return zq, loss, idx.astype(np.int32)
